# revision 16
# baseline (speedup 1.0000x reference)
"""GQA (B=2,S=1024,E=4096,H=32,KV=8,HD=128, RoPE, no causal mask) on 8 NeuronCores.

Sharding: 2 batch-groups x 4-way head tensor-parallel.
Core c: batch b=c//4, tp rank r=c%4 -> 8 q heads [8r,8r+8), 2 kv heads [2r,2r+2),
wo rows [1024r, 1024(r+1)).  Each core computes a partial output
out_part = y_local @ wo[local_rows, :]  (emitted transposed as [4096, 1024] fp16);
host sums the 4 partials per batch. No device collectives needed.

Attention phase uses V-stationary AV matmuls (512-row moving, LDWEIGHTS fully
hidden) producing y^T directly in the phase-E layout; the softmax denominator
is computed by a ones-matrix matmul that lands Z broadcast across all 128
partitions, so normalization is one vector multiply per head.  The Z/AV
matmuls of head h-1 are interleaved into head h's score slots so the PE never
waits on the scalar-engine exp stream.  The first superchunk of the input
projection is small so the opening matmul block is not gated behind 2MB of
queued DMA, and the first out-projection accumulation is folded into the
attention tail.
"""
import sys

sys.path.insert(0, "/opt/trn_rl_repo")

import numpy as np

B = 2
S = 1024
E = 4096
HD = 128
N_CORES = 8
TP = 4            # tensor-parallel ranks per batch group
HL = 8            # q heads per core
KVL = 2           # kv heads per core
QCOLS = HL * HD   # 1024
KVCOLS = KVL * HD  # 256
NCC = (QCOLS + 2 * KVCOLS) // 128  # 12 col-chunks of 128 (8 q, 2 k, 2 v)
ECH = E // 128    # 32 e-chunks
TT = S // 128     # 8 token tiles
SCALE = 1.0 / np.sqrt(np.float32(HD))
MM_DT = "float16"   # matmul operand dtype: "float16" or "float32r"


_PROGRAM = None


def _build_program():
    import concourse.bass as bass  # noqa: F401
    from concourse import bacc
    import concourse.mybir as mybir
    from concourse.tile import TileContext
    from concourse.masks import make_identity

    dt = mybir.dt.float32
    dtr = getattr(mybir.dt, MM_DT)
    nc = bacc.Bacc("TRN2", target_bir_lowering=False, debug=False,
                   num_devices=N_CORES)

    xt_d = nc.declare_dram_parameter("xt", [E, S], dtr, isOutput=False)
    wq_d = nc.declare_dram_parameter("wq", [E, QCOLS], dtr, isOutput=False)
    wk_d = nc.declare_dram_parameter("wk", [E, KVCOLS], dtr, isOutput=False)
    wv_d = nc.declare_dram_parameter("wv", [E, KVCOLS], dtr, isOutput=False)
    wo_d = nc.declare_dram_parameter("wo", [QCOLS, E], dtr, isOutput=False)
    cos_d = nc.declare_dram_parameter("cos", [HD, S], dtr, isOutput=False)
    sinp_d = nc.declare_dram_parameter("sinp", [HD, S], dtr, isOutput=False)
    out_d = nc.declare_dram_parameter("out_t", [E, S], dtr, isOutput=True)

    with TileContext(nc) as tc:
        with tc.tile_pool(name="const", bufs=1) as cpool, \
             tc.tile_pool(name="persist", bufs=1) as ppool, \
             tc.tile_pool(name="vnat", bufs=1) as vpool:
            ident_f = cpool.tile([128, 128], dt)
            make_identity(nc, ident_f[:])
            ident = cpool.tile([128, 128], dtr)
            nc.scalar.copy(ident[:], ident_f[:])
            ones_mat = cpool.tile([128, 128], dtr)
            nc.vector.memset(ones_mat[:], 1.0)
            expwarm = cpool.tile([1, 16], dtr)
            cos_t = cpool.tile([HD, S], dtr, tag="cos")
            sinp_t = cpool.tile([HD, S], dtr, tag="sinp")
            # persistent tiles: qkvT[cc] = [128 cols, S] transposed projections
            qkvT = [ppool.tile([128, S], dtr, tag=f"qkvT{i}", name=f"qkvT{i}") for i in range(NCC)]
            # yT[h] = [128 hd, S] transposed attention outputs
            yT = [ppool.tile([128, S], dtr, tag=f"yT{i}", name=f"yT{i}") for i in range(HL)]
            # v natural tiles: [128 k-tokens, HD]
            v_nat = [[vpool.tile([128, HD], dtr, tag=f"v{kv}_{kt}", name=f"v{kv}_{kt}")
                      for kt in range(TT)] for kv in range(KVL)]

            # ---------------- Phase A: QKV^T projections (x^T from host) ----------------
            # Superchunk sizes are staged small-to-large so the first matmul
            # block needs only 576KB of DMA, not 2.3MB.
            ES_SIZES = [2, 6, 8, 8, 8]
            NSUP = len(ES_SIZES)
            with tc.tile_pool(name="xsup", bufs=2) as xspool, \
                 tc.tile_pool(name="wstream", bufs=3) as wpool, \
                 tc.tile_pool(name="rope", bufs=3) as ropool, \
                 tc.tile_pool(name="psA", bufs=3, space="PSUM") as psA:
                CC_ORDER = [HL, HL + 1] + list(range(HL)) + [HL + KVL, HL + KVL + 1]
                # last superchunk: V first (feeds phase C immediately), then K
                # (roped first, needed by the first scores), then Q in order
                CC_LAST = [HL + KVL, HL + KVL + 1, HL, HL + 1] + list(range(HL))

                def w_src(eb, ecs, cc):
                    rows = slice(eb * 128, (eb + ecs) * 128)
                    if cc < HL:
                        return wq_d[rows, cc * 128:(cc + 1) * 128]
                    if cc < HL + KVL:
                        return wk_d[rows, (cc - HL) * 128:(cc - HL + 1) * 128]
                    return wv_d[rows, (cc - HL - KVL) * 128:(cc - HL - KVL + 1) * 128]

                eb = 0
                for si, ecs in enumerate(ES_SIZES):
                    last = (si == NSUP - 1)
                    cc_order = CC_LAST if last else CC_ORDER
                    # first weight tile before the xs chunks so the first
                    # matmul is not stuck behind queued DMAs
                    wt0 = wpool.tile([128, ecs, 128], dtr, tag="w", name=f"wt0_{si}")
                    nc.sync.dma_start(
                        out=wt0[:],
                        in_=w_src(eb, ecs, cc_order[0]).rearrange("(c p) m -> p c m", p=128))
                    xs = xspool.tile([128, ecs, S], dtr, tag="xs", name=f"xs{si}")
                    # x chunks go on the scalar-engine HWDGE queue so they never
                    # serialize behind the weight stream on the sync queue
                    for ec in range(ecs):
                        if si == 0 and ec == 0:
                            # halves: the first matmul only needs the first
                            # 512 tokens, so it starts one transfer earlier
                            for hf in range(2):
                                nc.scalar.dma_start(
                                    out=xs[:, 0, hf * 512:(hf + 1) * 512],
                                    in_=xt_d[eb * 128:(eb + 1) * 128,
                                             hf * 512:(hf + 1) * 512])
                            continue
                        if si == 0 and ec == 1:
                            # second chunk rides the sync queue so it is not
                            # stuck behind the first chunk's issue slots
                            nc.sync.dma_start(
                                out=xs[:, ec, :],
                                in_=xt_d[(eb + ec) * 128:(eb + ec + 1) * 128, :])
                            continue
                        nc.scalar.dma_start(
                            out=xs[:, ec, :],
                            in_=xt_d[(eb + ec) * 128:(eb + ec + 1) * 128, :])
                    if si == 1:
                        # warm the exp activation table while phase A runs so
                        # the first real exp is not delayed by the table load
                        # (after this superchunk's DMA issues, so the ~1.3us
                        # table load does not delay them)
                        nc.scalar.activation(expwarm[:], ident_f[0:1, 0:16],
                                             mybir.ActivationFunctionType.Exp,
                                             scale=1.0)
                    if si == 1:
                        nc.scalar.dma_start(out=cos_t[:], in_=cos_d[:])
                        nc.scalar.dma_start(out=sinp_t[:], in_=sinp_d[:])
                    for ci, cc in enumerate(cc_order):
                        if ci == 0:
                            wt = wt0
                        else:
                            wt = wpool.tile([128, ecs, 128], dtr, tag="w")
                            nc.sync.dma_start(
                                out=wt[:],
                                in_=w_src(eb, ecs, cc).rearrange("(c p) m -> p c m", p=128))
                        acc = psA.tile([128, S], dt, tag="acc")
                        for ec in range(ecs):
                            for tb in range(2):
                                nc.tensor.matmul(
                                    acc[:, tb * 512:(tb + 1) * 512], wt[:, ec, :],
                                    xs[:, ec, tb * 512:(tb + 1) * 512],
                                    start=(ec == 0), stop=(ec == ecs - 1))
                        if si == 0:
                            # vector, not scalar: the scalar queue is busy with
                            # the exp table preload and DMA issues here, and a
                            # scalar-copy chain would gate the psA buffer reuse
                            nc.vector.tensor_copy(qkvT[cc][:], acc[:])
                        else:
                            nc.vector.tensor_add(qkvT[cc][:], acc[:], qkvT[cc][:])
                        if last and cc < HL + KVL:
                            # rope immediately after the final accumulation of
                            # this chunk, overlapping remaining projections
                            sh = ropool.tile([HD, S], dtr, tag="sh")
                            nc.sync.dma_start(out=sh[0:64, :], in_=qkvT[cc][64:128, :])
                            nc.sync.dma_start(out=sh[64:128, :], in_=qkvT[cc][0:64, :])
                            t1 = ropool.tile([HD, S], dtr, tag="t1")
                            nc.vector.tensor_mul(t1[:], qkvT[cc][:], cos_t[:])
                            nc.vector.tensor_mul(sh[:], sh[:], sinp_t[:])
                            nc.vector.tensor_add(qkvT[cc][:], t1[:], sh[:])
                    eb += ecs

            # ---------------- Phase C: V natural ----------------
            with tc.tile_pool(name="psC", bufs=2, space="PSUM") as psC:
                for kv in range(KVL):
                    for kt in range(TT):
                        pt = psC.tile([128, 128], dtr, tag="ptC")
                        nc.tensor.transpose(
                            pt[:], qkvT[HL + KVL + kv][:, kt * 128:(kt + 1) * 128], ident[:])
                        nc.vector.tensor_copy(v_nat[kv][kt][:], pt[:])

            # ---------------- Phase D + E ----------------
            NOC = E // 128
            with tc.tile_pool(name="pT", bufs=20) as ptpool, \
                 tc.tile_pool(name="recs", bufs=3) as recpool, \
                 tc.tile_pool(name="wo", bufs=3) as wopool, \
                 tc.tile_pool(name="osb", bufs=3) as opool, \
                 tc.tile_pool(name="psP", bufs=2, space="PSUM") as psP, \
                 tc.tile_pool(name="psZ", bufs=1, space="PSUM") as psZ, \
                 tc.tile_pool(name="psY", bufs=1, space="PSUM") as psY:

                wt_e = {}
                op0 = None
                e_mms = []
                prev = None  # (head, pts) awaiting Z/AV/normalize
                for h in range(HL + 1):
                    if h < HL:
                        kv = h // (HL // KVL)
                        kT = qkvT[HL + kv]
                        cur_pts = []
                    else:
                        # attention tail: prefetch the first two wo column
                        # blocks (sync queue; the scalar queue is still
                        # draining exps) and open the first out-projection
                        # accumulation inside the tail slots
                        for oc in (0, 1):
                            wt_e[oc] = wopool.tile([128, HL, 128], dtr, tag="wo",
                                                   name=f"wtE{oc}")
                            nc.sync.dma_start(
                                out=wt_e[oc][:],
                                in_=wo_d[:, oc * 128:(oc + 1) * 128].rearrange(
                                    "(c p) m -> p c m", p=128))
                        op0 = psP.tile([128, S], dt, tag="sp", name="op0")
                        op1 = psP.tile([128, S], dt, tag="sp", name="op1")
                        ops = (op0, op1)
                        e_mms = [(oc, yc, tb) for oc in (0, 1)
                                 for yc in range(HL - 1) for tb in range(2)]
                    if prev is not None:
                        ph, ppts = prev
                        pkv = ph // (HL // KVL)
                        zb = psZ.tile([128, S], dt, tag="zb")
                        yp = psY.tile([128, S], dt, tag="yp")
                        recb = recpool.tile([128, S], dt, tag="recb")
                    for kc in range(TT):
                        if h < HL:
                            if h == 0 and kc == 0:
                                # psZ/psY banks are idle during head 0; borrow
                                # them so the opening scores are not gated by
                                # the exp drain of a 2-deep score rotation
                                sp = psZ.tile([128, S], dt, tag="zb", name="sp_w0")
                            elif h == 0 and kc == 1:
                                sp = psY.tile([128, S], dt, tag="yp", name="sp_w1")
                            else:
                                sp = psP.tile([128, S], dt, tag="sp")
                            for tb in range(2):
                                nc.tensor.matmul(
                                    sp[:, tb * 512:(tb + 1) * 512],
                                    kT[:, kc * 128:(kc + 1) * 128],
                                    qkvT[h][:, tb * 512:(tb + 1) * 512],
                                    start=True, stop=True)
                            pt = ptpool.tile([128, S], dtr, tag="pt")
                            nc.scalar.activation(pt[:], sp[:],
                                                 mybir.ActivationFunctionType.Exp,
                                                 scale=float(SCALE))
                            cur_pts.append(pt)
                        if prev is not None:
                            for tb in range(2):
                                nc.tensor.matmul(
                                    zb[:, tb * 512:(tb + 1) * 512], ones_mat[:],
                                    ppts[kc][:, tb * 512:(tb + 1) * 512],
                                    start=(kc == 0), stop=(kc == TT - 1))
                            if kc == TT - 1:
                                # tb0 reciprocal only; tb1 is interleaved with
                                # the tb0 normalize after the loop so the next
                                # head's AV is unblocked one DVE op sooner
                                nc.vector.reciprocal_approx_fast(
                                    out=recb[:, 0:512], in_=zb[:, 0:512])
                            for tb in range(2):
                                nc.tensor.matmul(
                                    yp[:, tb * 512:(tb + 1) * 512],
                                    v_nat[pkv][kc][:],
                                    ppts[kc][:, tb * 512:(tb + 1) * 512],
                                    start=(kc == 0), stop=(kc == TT - 1))
                        if h == HL:
                            for _ in range(2):
                                if e_mms:
                                    oc, yc, tb = e_mms.pop(0)
                                    nc.tensor.matmul(
                                        ops[oc][:, tb * 512:(tb + 1) * 512],
                                        wt_e[oc][:, yc, :],
                                        yT[yc][:, tb * 512:(tb + 1) * 512],
                                        start=(yc == 0), stop=False)
                    if prev is not None:
                        nc.vector.tensor_mul(
                            yT[ph][:, 0:512], yp[:, 0:512], recb[:, 0:512])
                        nc.vector.reciprocal_approx_fast(
                            out=recb[:, 512:1024], in_=zb[:, 512:1024])
                        nc.vector.tensor_mul(
                            yT[ph][:, 512:1024], yp[:, 512:1024],
                            recb[:, 512:1024])
                    prev = (h, cur_pts) if h < HL else None

                # ---------------- Phase E: out projection ----------------
                # finish the two accumulations opened in the attention tail
                for oc, yc, tb in e_mms:
                    nc.tensor.matmul(
                        ops[oc][:, tb * 512:(tb + 1) * 512], wt_e[oc][:, yc, :],
                        yT[yc][:, tb * 512:(tb + 1) * 512],
                        start=(yc == 0), stop=False)
                for oc in (0, 1):
                    for tb in range(2):
                        nc.tensor.matmul(
                            ops[oc][:, tb * 512:(tb + 1) * 512],
                            wt_e[oc][:, HL - 1, :],
                            yT[HL - 1][:, tb * 512:(tb + 1) * 512],
                            start=False, stop=True)
                for oc in range(NOC):
                    if oc <= 1:
                        op = ops[oc]
                    else:
                        op = psP.tile([128, S], dt, tag="sp", name=f"op{oc}")
                        if oc in wt_e:
                            wt = wt_e[oc]
                        else:
                            wt = wopool.tile([128, HL, 128], dtr, tag="wo")
                            nc.scalar.dma_start(
                                out=wt[:],
                                in_=wo_d[:, oc * 128:(oc + 1) * 128].rearrange(
                                    "(c p) m -> p c m", p=128))
                        for yc in range(HL):
                            for tb in range(2):
                                nc.tensor.matmul(
                                    op[:, tb * 512:(tb + 1) * 512], wt[:, yc, :],
                                    yT[yc][:, tb * 512:(tb + 1) * 512],
                                    start=(yc == 0), stop=(yc == HL - 1))
                    ot = opool.tile([128, S], dtr, tag="ot")
                    if oc >= NOC - 2:
                        # split the final tiles so the tail copy+store is short
                        for tb in range(2):
                            nc.scalar.copy(ot[:, tb * 512:(tb + 1) * 512],
                                           op[:, tb * 512:(tb + 1) * 512])
                            nc.sync.dma_start(
                                out=out_d[oc * 128:(oc + 1) * 128,
                                          tb * 512:(tb + 1) * 512],
                                in_=ot[:, tb * 512:(tb + 1) * 512])
                    else:
                        nc.scalar.copy(ot[:], op[:])
                        nc.sync.dma_start(
                            out=out_d[oc * 128:(oc + 1) * 128, :], in_=ot[:])

    nc.compile()
    return nc


def _rope_tables():
    inv = 1.0 / (10000.0 ** (np.arange(0, HD, 2, dtype=np.float32) / HD))  # [64]
    ang = np.arange(S, dtype=np.float32)[None, :] * inv[:, None]           # [64, S]
    cos = np.concatenate([np.cos(ang), np.cos(ang)], axis=0).astype(np.float32)   # [128, S]
    sin = np.sin(ang)
    sinp = np.concatenate([-sin, sin], axis=0).astype(np.float32)          # [128, S]
    return cos, sinp


def kernel(x, wq, wk, wv, wo):
    global _PROGRAM
    from concourse.bass_utils import run_bass_kernel_spmd

    if _PROGRAM is None:
        _PROGRAM = _build_program()
    nc = _PROGRAM

    cos, sinp = _rope_tables()
    ndt = np.float16 if MM_DT == "float16" else np.float32
    x = np.ascontiguousarray(x, dtype=np.float32)
    in_maps = []
    for c in range(N_CORES):
        b, r = c // TP, c % TP
        in_maps.append({
            "xt": np.ascontiguousarray(x[b].T).astype(ndt),
            "wq": np.ascontiguousarray(wq[:, r * QCOLS:(r + 1) * QCOLS], dtype=ndt),
            "wk": np.ascontiguousarray(wk[:, r * KVCOLS:(r + 1) * KVCOLS], dtype=ndt),
            "wv": np.ascontiguousarray(wv[:, r * KVCOLS:(r + 1) * KVCOLS], dtype=ndt),
            "wo": np.ascontiguousarray(wo[r * QCOLS:(r + 1) * QCOLS, :], dtype=ndt),
            "cos": cos.astype(ndt),
            "sinp": sinp.astype(ndt),
        })

    res = run_bass_kernel_spmd(nc, in_maps, list(range(N_CORES)))

    out = np.zeros((B, S, E), dtype=np.float32)
    for c in range(N_CORES):
        b = c // TP
        out[b] += res.results[c]["out_t"].astype(np.float32).T
    return out
